# revision 25
# baseline (speedup 1.0000x reference)
"""GaborConv2d Trainium2 kernel.

Strategy
--------
Host: generate the tiny [64,3,7,7] Gabor weights, pad the input, and build a
60-plane four-row im2col stack per image: output rows are processed in QUADS
(r=4q+rr, rr in 0..3); plane s=(dy*6 + c*2 + u) of quad q holds
pad[c, 4q+dy, u+x] for dy in 0..9, c in 0..2, u in 0..1. Even quads go to
partition group 0 (partitions 0..59), odd quads to group 1 (64..123).

Device (per core, 2 images batch-sharded): each matmul computes FOUR rows x
32 channels at once: stationary [60, 128] maps plane (dy,c,u) -> output
(rr*32+oc) with weight W[32h+oc, c, dy-rr, 2t+u] for channel half h; 4
accumulating supertaps t (kj=2t+u) with moving [60, 512] read at free
offset 2t. K=60 <= 64 keeps each matmul on a 64-row PE tile; consecutive
matmuls strictly alternate tile_position row-groups (0,0)/(64,0) so two
matmuls overlap on disjoint PE halves (~2x throughput; same-position
back-to-back matmuls measure 3.7x slower). The 4-row packing cuts im2col
HBM traffic to 15 plane-rows per output row. PSUM: two banks per quad, 4
quads in flight; sub-sweeps of 2 quads iterate t-outer; while quads 2-3
compute, quads 0-1 are evicted (f32->bf16) by VectorE/ScalarE into a
staging tile. Stores are 1MB DMAs per 8 quads per half (8KB/partition
lines). Output DRAM layout is [img, half, rr, oc, quad, x]; host
reassembles rows/channels at the end.
"""

import math

import ml_dtypes
import numpy as np

import concourse.bass as bass
import concourse.mybir as mybir
import concourse.tile as tile
from concourse import bacc
from concourse.bass_utils import run_bass_kernel_spmd

F32 = mybir.dt.float32
BF16 = mybir.dt.bfloat16
BF16NP = ml_dtypes.bfloat16

N_CORES = 8
B, C, H, W = 16, 3, 512, 512
O, K, PAD = 64, 7, 3
IPC = B // N_CORES          # images per core
NP = 60                     # planes: dy(10) * c(3) * u(2)
XW = 520                    # stored plane width
NQUAD = H // 4              # 128 row quads per image
QB = 16                     # quads per block (8 per parity group)
NT = 4                      # supertaps, kj = 2t+u
HPAD = H + 2 * PAD          # 518 padded rows
WPAD = 524                  # padded width (3 + 512 + 3, +u slack, even)
DELTA = 0.001


def _gabor_weights(freq, theta, sigma, psi):
    x0 = math.ceil(K / 2)
    lin = np.linspace(-x0 + 1, x0, K, dtype=np.float32)
    y = np.broadcast_to(lin[:, None], (K, K))
    x = np.broadcast_to(lin[None, :], (K, K))
    th = theta[:, :, None, None].astype(np.float32)
    fr = freq[:, :, None, None].astype(np.float32)
    sg = sigma[:, :, None, None].astype(np.float32)
    ps = psi[:, :, None, None].astype(np.float32)
    rotx = x * np.cos(th) + y * np.sin(th)
    roty = -x * np.sin(th) + y * np.cos(th)
    g = np.exp(-0.5 * ((rotx**2 + roty**2) / (sg + DELTA) ** 2))
    g = g * np.cos(fr * rotx + ps)
    g = g / (2 * np.pi * sg**2)
    return g.astype(np.float32)  # [O, C, K, K]


def _build_nc():
    nc = bacc.Bacc(None, target_bir_lowering=False)
    # (img, parity, plane, quadcol, x)
    xs = nc.dram_tensor("xstack", [IPC, 2, NP, NQUAD // 2, XW], BF16,
                        kind="ExternalInput")
    wb = nc.dram_tensor("wbig", [128, NT * 2 * 128], BF16, kind="ExternalInput")
    # (img, half, rr, oc, quad, x)
    y = nc.dram_tensor("y", [IPC, 2, 4, O // 2, NQUAD, W], BF16,
                       kind="ExternalOutput")

    QCB = QB // 2  # quadcols per group per block

    with tile.TileContext(nc) as tc:
        with (
            tc.tile_pool(name="wpool", bufs=1) as wpool,
            tc.tile_pool(name="ipool", bufs=4) as ipool,
            tc.tile_pool(name="spool", bufs=3) as spool,
            tc.tile_pool(name="ppool", bufs=8, space="PSUM") as ppool,
        ):
            wt = wpool.tile([128, NT * 2 * 128], BF16)
            nc.sync.dma_start(out=wt, in_=wb[:])

            for img in range(IPC):
                for blk in range(NQUAD // QB):
                    it = ipool.tile([128, QCB * XW], BF16, tag="img")
                    first = img == 0 and blk == 0
                    for gg in range(2):
                        # split the very first load so matmul 0 starts sooner
                        chunks = ((0, 2), (2, QCB - 2)) if first else ((0, QCB),)
                        for qc0, qcn in chunks:
                            nc.scalar.dma_start(
                                out=it[64 * gg : 64 * gg + NP,
                                       qc0 * XW : (qc0 + qcn) * XW],
                                in_=bass.AP(
                                    xs,
                                    ((img * 2 + gg) * NP) * (NQUAD // 2) * XW
                                    + (blk * QCB + qc0) * XW,
                                    [[(NQUAD // 2) * XW, NP], [1, qcn * XW]],
                                ),
                            )
                    stg = spool.tile([128, QB * 2 * W], BF16, tag="stg")
                    for quarter in range(QB // 4):
                        for sub in range(2):  # 2-quad sub-sweeps, 4 banks
                            pss = [
                                ppool.tile([128, W], F32, tag="ps", name=f"ps{b}")
                                for b in range(4)
                            ]
                            for t in range(NT):
                                for h in range(2):
                                    for qq in range(2):  # group alternates
                                        qloc = quarter * 4 + sub * 2 + qq
                                        b = qq * 2 + h
                                        gg = qloc % 2
                                        qcol = qloc // 2
                                        nc.tensor.matmul(
                                            pss[b][:, :],
                                            wt[64 * gg : 64 * gg + NP,
                                               (t * 2 + h) * 128
                                               : (t * 2 + h + 1) * 128],
                                            it[64 * gg : 64 * gg + NP,
                                               qcol * XW + 2 * t
                                               : qcol * XW + 2 * t + W],
                                            start=(t == 0),
                                            stop=(t == NT - 1),
                                        )
                            for b in range(4):
                                qloc = quarter * 4 + sub * 2 + b // 2
                                h = b % 2
                                sl = stg[:, (h * QB + qloc) * W
                                         : (h * QB + qloc + 1) * W]
                                if b % 2 == 0:
                                    nc.vector.tensor_copy(sl, pss[b][:, :])
                                else:
                                    nc.scalar.copy(sl, pss[b][:, :])
                    last = img == IPC - 1 and blk == NQUAD // QB - 1
                    segn = 4 if last else 8  # finer final stores shrink tail
                    for seg in range(QB // segn):
                        for h in range(2):
                            # partitions: rr*32+oc ; stg cols (h*QB+qloc)*W
                            nc.sync.dma_start(
                                out=bass.AP(
                                    y,
                                    img * (2 * 4 * (O // 2) * NQUAD * W)
                                    + h * (4 * (O // 2) * NQUAD * W)
                                    + (blk * QB + seg * segn) * W,
                                    [[NQUAD * W, 128], [1, segn * W]],
                                ),
                                in_=stg[:, (h * QB + seg * segn) * W
                                        : (h * QB + seg * segn + segn) * W],
                            )
    nc.finalize()
    return nc


def _prepare_inputs(input_tensor, freq, theta, sigma, psi):
    g = _gabor_weights(freq, theta, sigma, psi)  # [O, C, K, K] f32
    # wbig[64*grp + dy*6+c*2+u, (t*2+h)*128 + rr*32 + oc]
    #   = g[32h+oc, c, dy-rr, 2t+u]
    wmat = np.zeros((128, NT * 2 * 128), np.float32)
    for t in range(NT):
        for h in range(2):
            for dy in range(10):
                for c in range(C):
                    for u in range(2):
                        kj = 2 * t + u
                        if kj >= K:
                            continue
                        s = dy * 6 + c * 2 + u
                        for rr in range(4):
                            ki = dy - rr
                            if not (0 <= ki < K):
                                continue
                            col = (t * 2 + h) * 128 + rr * 32
                            wmat[s, col : col + 32] = g[32 * h : 32 * h + 32,
                                                        c, ki, kj]
                            wmat[64 + s, col : col + 32] = wmat[s, col : col + 32]
    wbig = wmat.astype(BF16NP)

    xb = input_tensor.astype(BF16NP)
    pad = np.zeros((B, C, HPAD, WPAD), BF16NP)
    pad[:, :, PAD : PAD + H, PAD : PAD + W] = xb
    # xstack[img, gg, dy*6+c*2+u, qc, x] = pad[img, c, 4*(2qc+gg)+dy, u+x]
    in_maps = []
    for core in range(N_CORES):
        imgs = pad[core * IPC : (core + 1) * IPC]
        xstack = np.empty((IPC, 2, NP, NQUAD // 2, XW), BF16NP)
        for gg in range(2):
            for dy in range(10):
                for c in range(C):
                    for u in range(2):
                        s = dy * 6 + c * 2 + u
                        r0 = 4 * gg + dy
                        xstack[:, gg, s] = imgs[
                            :, c, r0 : r0 + 8 * (NQUAD // 2 - 1) + 1 : 8,
                            u : u + XW
                        ]
        in_maps.append({"xstack": xstack, "wbig": wbig})
    return in_maps


_NC_CACHE = None


def kernel(input_tensor, freq, theta, sigma, psi):
    global _NC_CACHE
    input_tensor = np.asarray(input_tensor, dtype=np.float32)
    in_maps = _prepare_inputs(
        input_tensor,
        np.asarray(freq), np.asarray(theta), np.asarray(sigma), np.asarray(psi),
    )
    if _NC_CACHE is None:
        _NC_CACHE = _build_nc()
    res = run_bass_kernel_spmd(_NC_CACHE, in_maps, core_ids=list(range(N_CORES)))
    out = np.empty((B, O, H, W), np.float32)
    for core in range(N_CORES):
        ydev = res.results[core]["y"]  # [IPC, 2, 4, 32, NQUAD, W] bf16
        for i in range(IPC):
            img = core * IPC + i
            for h in range(2):
                for rr in range(4):
                    out[img, 32 * h : 32 * h + 32, rr::4, :] = ydev[i, h, rr]
    return out
